# revision 5
# baseline (speedup 1.0000x reference)
"""NetVLAD Trainium2 Bass kernel.

Full inputs in, full output out. Data-parallel over batch N=64 across 8
NeuronCores (8 samples per core); conv weight and centroids replicated.

Per-sample algorithm (mathematically equal to the reference, never
materializing the channel-normalized x):
  X = x[n]  [D=128, P=4800]  (D on SBUF partitions, contiguous in HBM)
  For each 128-wide chunk of P (p on partitions after a PE transpose):
    ss[p]    = sum_d X[d,p]^2            (ACT Square with accum_out)
    inv_s    = exp(-0.5*ln(ss))          (batched; rsqrt without the
                                          sqrt table set: Ln/Exp/Square
                                          all live in one ACT table set)
    logitsT  = X_c^T @ Wt                (PE, shares lhsT with transpose)
    e        = exp(logitsT * inv_s)      (ACT, per-partition scale,
                                          accum_out gives Z; softmax
                                          max-subtraction is skipped:
                                          |logits*inv_s| <= ~1.2)
    sb       = e * (inv_s / Z)
    acc     += [sb | e]^T @ [X_c^T | 1/Z]   (PE, PSUM accumulate)
  agg      = acc[0:64, 0:128];  sum_sa = acc[64:128, 128]
  vlad     = agg - sum_sa * centroids, then intra + global L2 norm.
"""

import sys

if "/opt/trn_rl_repo" not in sys.path:
    sys.path.insert(0, "/opt/trn_rl_repo")

import numpy as np
from contextlib import ExitStack

N, D, HW, K = 64, 128, 4800, 64
NCORES = 8
NS = N // NCORES  # samples per core

CHUNKS = [(i * 128, min(128, HW - i * 128)) for i in range((HW + 127) // 128)]
NCH = len(CHUNKS)  # 38: 37 full + one 64-wide

_CACHE = {}


def _build_nc():
    import concourse.tile as tile
    from concourse import bacc, mybir

    nc = bacc.Bacc(
        "TRN2",
        target_bir_lowering=False,
        debug=False,
        enable_asserts=False,
        num_devices=NCORES,
    )
    x_ap = nc.dram_tensor("x", [NS, D, HW], mybir.dt.float32, kind="ExternalInput").ap()
    wt_ap = nc.dram_tensor("wt", [D, K], mybir.dt.float32, kind="ExternalInput").ap()
    cent_ap = nc.dram_tensor(
        "cent", [K, D], mybir.dt.float32, kind="ExternalInput"
    ).ap()
    out_ap = nc.dram_tensor(
        "out", [NS, K, D], mybir.dt.float32, kind="ExternalOutput"
    ).ap()

    with tile.TileContext(nc) as tc:
        with ExitStack() as ctx:
            _body(ctx, tc, out_ap, x_ap, wt_ap, cent_ap)
    nc.compile()
    return nc


def _body(ctx, tc, out_ap, x_ap, wt_ap, cent_ap):
    import concourse.bass as bass
    from concourse import masks, mybir

    nc = tc.nc
    f32 = mybir.dt.float32
    AF = mybir.ActivationFunctionType
    ALU = mybir.AluOpType

    singles = ctx.enter_context(tc.tile_pool(name="singles", bufs=1))
    xpool = ctx.enter_context(tc.tile_pool(name="xpool", bufs=2))
    xtrpool = ctx.enter_context(tc.tile_pool(name="xtrpool", bufs=2))
    ebpool = ctx.enter_context(tc.tile_pool(name="ebpool", bufs=2))
    lpool = ctx.enter_context(tc.tile_pool(name="lpool", bufs=2))
    scrpool = ctx.enter_context(tc.tile_pool(name="scrpool", bufs=2))
    smalls = ctx.enter_context(tc.tile_pool(name="smalls", bufs=4))
    tails = ctx.enter_context(tc.tile_pool(name="tails", bufs=2))
    tsml = ctx.enter_context(tc.tile_pool(name="tsml", bufs=4))
    pp_xt = ctx.enter_context(tc.tile_pool(name="pp_xt", bufs=2, space="PSUM"))
    pp_lg = ctx.enter_context(tc.tile_pool(name="pp_lg", bufs=2, space="PSUM"))
    pp_acc = ctx.enter_context(tc.tile_pool(name="pp_acc", bufs=2, space="PSUM"))
    pp_tiny = ctx.enter_context(tc.tile_pool(name="pp_tiny", bufs=1, space="PSUM"))

    # constants
    ident = singles.tile([128, 128], f32)
    masks.make_identity(nc, ident[:])
    wt_s = singles.tile([D, K], f32)
    nc.sync.dma_start(out=wt_s[:], in_=wt_ap[:])
    cent_s = singles.tile([K, D], f32)
    nc.sync.dma_start(out=cent_s[:], in_=cent_ap[:])
    ones_col = singles.tile([K, 1], f32)
    nc.vector.memset(ones_col[:], 1.0)
    ones_row = singles.tile([1, K], f32)
    nc.vector.memset(ones_row[:], 1.0)

    for n in range(NS):
        # whole-sample load: one big DMA, 2.46 MB, 128 partitions
        xs = xpool.tile([D, HW], f32, tag="xs")
        nc.sync.dma_start(out=xs[:], in_=x_ap[n])

        # [XT | r] per chunk: cols 0:128 = X_c^T, col 128 = 1/Z
        xtr = xtrpool.tile([128, NCH, 129], f32, tag="xtr")
        # [sb | e] per chunk
        eb = ebpool.tile([128, NCH, 128], f32, tag="eb")
        # logits stash
        lgs = lpool.tile([128, NCH, K], f32, tag="lgs")

        ss = smalls.tile([128, NCH], f32, tag="ss")
        zz = smalls.tile([128, NCH], f32, tag="zz")
        is_ = smalls.tile([128, NCH], f32, tag="is")
        lns = smalls.tile([128, NCH], f32, tag="lns")
        # garbage lanes of the ragged last chunk must stay finite
        nc.vector.memset(ss[:, NCH - 1 : NCH], 1.0)
        nc.vector.memset(zz[:, NCH - 1 : NCH], 1.0)

        # ---- pass A: transpose + logits + sumsq ----
        for c, (p0, w) in enumerate(CHUNKS):
            x_c = xs[:, p0 : p0 + w]
            xt_p = pp_xt.tile([128, 128], f32, tag="xt")
            nc.tensor.transpose(xt_p[:w, :], x_c, ident[:])
            lg_p = pp_lg.tile([128, K], f32, tag="lg")
            nc.tensor.matmul(lg_p[:w, :], lhsT=x_c, rhs=wt_s[:], start=True, stop=True)
            nc.vector.tensor_copy(xtr[:w, c, 0:128], xt_p[:w, :])
            nc.vector.tensor_copy(lgs[:w, c, :], lg_p[:w, :])
            scr = scrpool.tile([128, 128], f32, tag="scr")
            nc.scalar.activation(
                scr[:w, :], xt_p[:w, :], AF.Square, accum_out=ss[:w, c : c + 1]
            )

        # inv_s = exp(-0.5 * ln(ss)), batched over all chunks
        nc.scalar.activation(lns[:, :], ss[:, :], AF.Ln)
        nc.scalar.activation(is_[:, :], lns[:, :], AF.Exp, scale=-0.5)

        # ---- pass B1: exp ----
        for c, (p0, w) in enumerate(CHUNKS):
            nc.scalar.activation(
                eb[:w, c, K : 2 * K],
                lgs[:w, c, :],
                AF.Exp,
                scale=is_[:w, c : c + 1],
                accum_out=zz[:w, c : c + 1],
            )

        # r = 1/Z for all chunks, written into col 128 of each xtr chunk
        r_view = xtr[:, :, 128]
        nc.vector.reciprocal(r_view, zz[:, :])

        # ---- pass B2: sb + accumulate matmul ----
        acc_p = pp_acc.tile([128, 129], f32, tag="acc")
        for c, (p0, w) in enumerate(CHUNKS):
            nc.vector.tensor_scalar(
                out=eb[:w, c, 0:K],
                in0=eb[:w, c, K : 2 * K],
                scalar1=is_[:w, c : c + 1],
                scalar2=xtr[:w, c, 128:129],
                op0=ALU.mult,
                op1=ALU.mult,
            )
            nc.tensor.matmul(
                acc_p[:, :],
                lhsT=eb[:w, c, :],
                rhs=xtr[:w, c, :],
                start=(c == 0),
                stop=(c == NCH - 1),
            )

        # ---- tail: vlad assembly + norms ----
        ssa = tsml.tile([K, 1], f32, tag="ssa")
        nc.vector.tensor_copy(ssa[:], acc_p[K : 2 * K, 128:129])
        tmp = tails.tile([K, D], f32, tag="t_tmp")
        nc.vector.tensor_scalar_mul(tmp[:], cent_s[:], ssa[:])
        vl = tails.tile([K, D], f32, tag="t_vl")
        nc.vector.tensor_tensor(
            out=vl[:], in0=acc_p[0:K, 0:D], in1=tmp[:], op=ALU.subtract
        )

        q = tsml.tile([K, 1], f32, tag="q")
        scr2 = tails.tile([K, D], f32, tag="t_scr")
        nc.scalar.activation(scr2[:], vl[:], AF.Square, accum_out=q[:])
        qm = tsml.tile([K, 1], f32, tag="qm")
        nc.vector.tensor_scalar_max(qm[:], q[:], 1e-24)
        lq = tsml.tile([K, 1], f32, tag="lq")
        nc.scalar.activation(lq[:], qm[:], AF.Ln)
        isq = tsml.tile([K, 1], f32, tag="isq")
        nc.scalar.activation(isq[:], lq[:], AF.Exp, scale=-0.5)
        vn = tails.tile([K, D], f32, tag="t_vn")
        nc.vector.tensor_scalar_mul(vn[:], vl[:], isq[:])

        g1 = tsml.tile([K, 1], f32, tag="g1")
        nc.scalar.activation(scr2[:], vn[:], AF.Square, accum_out=g1[:])
        gp_full = pp_tiny.tile([K, 1], f32, tag="tiny")
        gp = gp_full[0:1, :]
        nc.tensor.matmul(gp[:], lhsT=g1[:], rhs=ones_col[:], start=True, stop=True)
        gs0 = tsml.tile([1, 1], f32, tag="gs0")
        nc.vector.tensor_copy(gs0[:], gp[:])
        gm = tsml.tile([1, 1], f32, tag="gm")
        nc.vector.tensor_scalar_max(gm[:], gs0[:], 1e-24)
        lg2 = tsml.tile([1, 1], f32, tag="lg2")
        nc.scalar.activation(lg2[:], gm[:], AF.Ln)
        gis = tsml.tile([1, 1], f32, tag="gis")
        nc.scalar.activation(gis[:], lg2[:], AF.Exp, scale=-0.5)
        gb_p = pp_tiny.tile([K, 1], f32, tag="tiny")
        nc.tensor.matmul(gb_p[:], lhsT=ones_row[:], rhs=gis[:], start=True, stop=True)
        gb = tsml.tile([K, 1], f32, tag="gb")
        nc.vector.tensor_copy(gb[:], gb_p[:])
        vf = tails.tile([K, D], f32, tag="t_vf")
        nc.vector.tensor_scalar_mul(vf[:], vn[:], gb[:])
        nc.sync.dma_start(out=out_ap[n], in_=vf[:])


def kernel(x, conv_w, centroids):
    from concourse.bass_utils import run_bass_kernel_spmd

    if "nc" not in _CACHE:
        _CACHE["nc"] = _build_nc()
    nc = _CACHE["nc"]

    x = np.ascontiguousarray(np.asarray(x, dtype=np.float32).reshape(N, D, HW))
    wt = np.ascontiguousarray(np.asarray(conv_w, dtype=np.float32).T)
    cent = np.ascontiguousarray(np.asarray(centroids, dtype=np.float32))
    in_maps = [
        {"x": x[i * NS : (i + 1) * NS], "wt": wt, "cent": cent} for i in range(NCORES)
    ]
    res = run_bass_kernel_spmd(nc, in_maps, core_ids=list(range(NCORES))).results
    out = np.concatenate([r["out"].reshape(NS, K * D) for r in res], axis=0)
    return out


if __name__ == "__main__":
    rng = np.random.default_rng(0)
    xs = rng.standard_normal((N, D, 60, 80), dtype=np.float32)
    cw = (rng.standard_normal((K, D)) * 0.1).astype(np.float32)
    ct = rng.random((K, D), dtype=np.float32)
    o = kernel(x=xs, conv_w=cw, centroids=ct)
    print("kernel out", o.shape, o.dtype, np.abs(o).max())


# revision 7
# speedup vs baseline: 1.6983x; 1.6983x over previous
"""NetVLAD Trainium2 Bass kernel.

Full inputs in, full output out. Data-parallel over batch N=64 across 8
NeuronCores (8 samples per core); conv weight and centroids replicated.

Per-sample algorithm (mathematically equal to the reference, never
materializing the channel-normalized x):
  X = x[n]  [D=128, P=4800]  (D on SBUF partitions, contiguous in HBM)
  For each 128-wide chunk of P (p on partitions after a PE transpose):
    ss[p]    = sum_d X[d,p]^2            (ACT Square with accum_out)
    inv_s    = exp(-0.5*ln(ss))          (batched; rsqrt without the
                                          sqrt table set: Ln/Exp/Square
                                          all live in one ACT table set)
    logitsT  = X_c^T @ Wt                (PE, shares lhsT with transpose)
    e        = exp(logitsT * inv_s)      (ACT, per-partition scale,
                                          accum_out gives Z; softmax
                                          max-subtraction is skipped:
                                          |logits*inv_s| <= ~1.2)
    sb       = e * (inv_s / Z)
    acc     += [sb | e]^T @ [X_c^T | 1/Z]   (PE, PSUM accumulate)
  agg      = acc[0:64, 0:128];  sum_sa = acc[64:128, 128]
  vlad     = agg - sum_sa * centroids, then intra + global L2 norm.
"""

import sys

if "/opt/trn_rl_repo" not in sys.path:
    sys.path.insert(0, "/opt/trn_rl_repo")

import numpy as np
from contextlib import ExitStack

N, D, HW, K = 64, 128, 4800, 64
NCORES = 8
NS = N // NCORES  # samples per core

CHUNKS = [(i * 128, min(128, HW - i * 128)) for i in range((HW + 127) // 128)]
NCH = len(CHUNKS)  # 38: 37 full + one 64-wide

_CACHE = {}


def _build_nc():
    import concourse.tile as tile
    from concourse import bacc, mybir

    nc = bacc.Bacc(
        "TRN2",
        target_bir_lowering=False,
        debug=False,
        enable_asserts=False,
        num_devices=NCORES,
    )
    x_ap = nc.dram_tensor("x", [NS, D, HW], mybir.dt.float32, kind="ExternalInput").ap()
    wt_ap = nc.dram_tensor("wt", [D, K], mybir.dt.float32, kind="ExternalInput").ap()
    cent_ap = nc.dram_tensor(
        "cent", [K, D], mybir.dt.float32, kind="ExternalInput"
    ).ap()
    out_ap = nc.dram_tensor(
        "out", [NS, K, D], mybir.dt.float32, kind="ExternalOutput"
    ).ap()

    with tile.TileContext(nc) as tc:
        with ExitStack() as ctx:
            _body(ctx, tc, out_ap, x_ap, wt_ap, cent_ap)
    nc.compile()
    return nc


def _body(ctx, tc, out_ap, x_ap, wt_ap, cent_ap):
    import concourse.bass as bass
    from concourse import masks, mybir

    nc = tc.nc
    f32 = mybir.dt.float32
    AF = mybir.ActivationFunctionType
    ALU = mybir.AluOpType

    singles = ctx.enter_context(tc.tile_pool(name="singles", bufs=1))
    xpool = ctx.enter_context(tc.tile_pool(name="xpool", bufs=2))
    xtrpool = ctx.enter_context(tc.tile_pool(name="xtrpool", bufs=2))
    ebpool = ctx.enter_context(tc.tile_pool(name="ebpool", bufs=2))
    lpool = ctx.enter_context(tc.tile_pool(name="lpool", bufs=2))
    scrpool = ctx.enter_context(tc.tile_pool(name="scrpool", bufs=1))
    smalls = ctx.enter_context(tc.tile_pool(name="smalls", bufs=2))
    tails = ctx.enter_context(tc.tile_pool(name="tails", bufs=2))
    tsml = ctx.enter_context(tc.tile_pool(name="tsml", bufs=4))
    pp_xt = ctx.enter_context(tc.tile_pool(name="pp_xt", bufs=2, space="PSUM"))
    pp_lg = ctx.enter_context(tc.tile_pool(name="pp_lg", bufs=2, space="PSUM"))
    pp_acc = ctx.enter_context(tc.tile_pool(name="pp_acc", bufs=2, space="PSUM"))
    pp_tiny = ctx.enter_context(tc.tile_pool(name="pp_tiny", bufs=1, space="PSUM"))

    def bcast(ap, n):
        # append a step-0 free dim: [..., n] broadcast view
        return bass.AP(tensor=ap.tensor, offset=ap.offset, ap=list(ap.ap) + [[0, n]])

    # constants
    ident = singles.tile([128, 128], f32)
    masks.make_identity(nc, ident[:])
    wt_s = singles.tile([D, K], f32)
    nc.sync.dma_start(out=wt_s[:], in_=wt_ap[:])
    cent_s = singles.tile([K, D], f32)
    nc.sync.dma_start(out=cent_s[:], in_=cent_ap[:])
    ones_col = singles.tile([K, 1], f32)
    nc.vector.memset(ones_col[:], 1.0)
    ones_row = singles.tile([1, K], f32)
    nc.vector.memset(ones_row[:], 1.0)

    GRP = 4  # transpose chunks per PSUM bank group
    groups = []
    c0 = 0
    while c0 < NCH:
        groups.append(list(range(c0, min(c0 + GRP, NCH))))
        c0 += GRP

    for n in range(NS):
        # whole-sample load: one big DMA, 2.46 MB, 128 partitions
        xs = xpool.tile([D, HW], f32, tag="xs")
        nc.sync.dma_start(out=xs[:], in_=x_ap[n])

        # [XT | r] per chunk: cols 0:128 = X_c^T, col 128 = 1/Z
        xtr = xtrpool.tile([128, NCH, 129], f32, tag="xtr")
        # [sb | e] per chunk (cols 0:64 hold scaled logits, then sb)
        eb = ebpool.tile([128, NCH, 128], f32, tag="eb")
        # logits stash
        lgs = lpool.tile([128, NCH, K], f32, tag="lgs")
        # squared-XT scratch
        x2t = scrpool.tile([128, NCH * 128], f32, tag="x2t")

        ss = smalls.tile([128, NCH], f32, tag="ss")
        zz = smalls.tile([128, NCH], f32, tag="zz")
        is_ = smalls.tile([128, NCH], f32, tag="is")
        lns = smalls.tile([128, NCH], f32, tag="lns")
        tsc = smalls.tile([128, NCH], f32, tag="tsc")

        # ---- pass A: transpose + logits, evacuated per 4-chunk group ----
        for grp in groups:
            gn = len(grp)
            xt_p = pp_xt.tile([128, GRP * 128], f32, tag="xt")
            lg_p = pp_lg.tile([128, GRP * K], f32, tag="lg")
            for j, c in enumerate(grp):
                p0, w = CHUNKS[c]
                x_c = xs[:, p0 : p0 + w]
                nc.tensor.transpose(
                    xt_p[:w, j * 128 : j * 128 + 128], x_c, ident[:]
                )
                nc.tensor.matmul(
                    lg_p[:w, j * K : (j + 1) * K],
                    lhsT=x_c,
                    rhs=wt_s[:],
                    start=True,
                    stop=True,
                )
            gc = grp[0]
            src_xt = xt_p[:, 0 : gn * 128].rearrange("p (c d) -> p c d", c=gn)
            nc.vector.tensor_copy(xtr[:, gc : gc + gn, 0:128], src_xt)
            src_lg = lg_p[:, 0 : gn * K].rearrange("p (c k) -> p c k", c=gn)
            nc.scalar.copy(lgs[:, gc : gc + gn, :], src_lg)

        # ---- batched per-sample scalar pipeline ----
        # ss = rowsum(XT^2) per chunk; inv_s = exp(-0.5*ln(ss))
        nc.scalar.activation(
            x2t[:].rearrange("p (c d) -> p c d", c=NCH), xtr[:, :, 0:128], AF.Square
        )
        nc.vector.tensor_reduce(
            out=ss[:],
            in_=x2t[:].rearrange("p (c d) -> p c d", c=NCH),
            axis=mybir.AxisListType.X,
            op=ALU.add,
        )
        nc.scalar.activation(lns[:], ss[:], AF.Ln)
        nc.scalar.activation(is_[:], lns[:], AF.Exp, scale=-0.5)

        # scaled logits -> eb[:,:,0:64]; e = exp -> eb[:,:,64:128]
        nc.gpsimd.tensor_tensor(
            out=eb[:, :, 0:K], in0=lgs[:], in1=bcast(is_[:], K), op=ALU.mult
        )
        nc.scalar.activation(eb[:, :, K : 2 * K], eb[:, :, 0:K], AF.Exp)
        nc.vector.tensor_reduce(
            out=zz[:], in_=eb[:, :, K : 2 * K], axis=mybir.AxisListType.X, op=ALU.add
        )
        # r = 1/Z into col 128 of each xtr chunk; t = inv_s * r
        nc.vector.reciprocal(xtr[:, :, 128], zz[:])
        nc.vector.tensor_tensor(
            out=tsc[:], in0=is_[:], in1=xtr[:, :, 128], op=ALU.mult
        )
        # sb = e * t -> eb[:,:,0:64]
        nc.gpsimd.tensor_tensor(
            out=eb[:, :, 0:K],
            in0=eb[:, :, K : 2 * K],
            in1=bcast(tsc[:], K),
            op=ALU.mult,
        )

        # ---- pass C: accumulate matmuls ----
        acc_p = pp_acc.tile([128, 129], f32, tag="acc")
        for c, (p0, w) in enumerate(CHUNKS):
            nc.tensor.matmul(
                acc_p[:, :],
                lhsT=eb[:w, c, :],
                rhs=xtr[:w, c, :],
                start=(c == 0),
                stop=(c == NCH - 1),
            )

        # ---- tail: vlad assembly + norms ----
        ssa = tsml.tile([K, 1], f32, tag="ssa")
        nc.vector.tensor_copy(ssa[:], acc_p[K : 2 * K, 128:129])
        tmp = tails.tile([K, D], f32, tag="t_tmp")
        nc.vector.tensor_scalar_mul(tmp[:], cent_s[:], ssa[:])
        vl = tails.tile([K, D], f32, tag="t_vl")
        nc.vector.tensor_tensor(
            out=vl[:], in0=acc_p[0:K, 0:D], in1=tmp[:], op=ALU.subtract
        )

        q = tsml.tile([K, 1], f32, tag="q")
        scr2 = tails.tile([K, D], f32, tag="t_scr")
        nc.scalar.activation(scr2[:], vl[:], AF.Square, accum_out=q[:])
        qm = tsml.tile([K, 1], f32, tag="qm")
        nc.vector.tensor_scalar_max(qm[:], q[:], 1e-24)
        lq = tsml.tile([K, 1], f32, tag="lq")
        nc.scalar.activation(lq[:], qm[:], AF.Ln)
        isq = tsml.tile([K, 1], f32, tag="isq")
        nc.scalar.activation(isq[:], lq[:], AF.Exp, scale=-0.5)
        vn = tails.tile([K, D], f32, tag="t_vn")
        nc.vector.tensor_scalar_mul(vn[:], vl[:], isq[:])

        g1 = tsml.tile([K, 1], f32, tag="g1")
        nc.scalar.activation(scr2[:], vn[:], AF.Square, accum_out=g1[:])
        gp_full = pp_tiny.tile([K, 1], f32, tag="tiny")
        gp = gp_full[0:1, :]
        nc.tensor.matmul(gp[:], lhsT=g1[:], rhs=ones_col[:], start=True, stop=True)
        gs0 = tsml.tile([1, 1], f32, tag="gs0")
        nc.vector.tensor_copy(gs0[:], gp[:])
        gm = tsml.tile([1, 1], f32, tag="gm")
        nc.vector.tensor_scalar_max(gm[:], gs0[:], 1e-24)
        lg2 = tsml.tile([1, 1], f32, tag="lg2")
        nc.scalar.activation(lg2[:], gm[:], AF.Ln)
        gis = tsml.tile([1, 1], f32, tag="gis")
        nc.scalar.activation(gis[:], lg2[:], AF.Exp, scale=-0.5)
        gb_p = pp_tiny.tile([K, 1], f32, tag="tiny")
        nc.tensor.matmul(gb_p[:], lhsT=ones_row[:], rhs=gis[:], start=True, stop=True)
        gb = tsml.tile([K, 1], f32, tag="gb")
        nc.vector.tensor_copy(gb[:], gb_p[:])
        vf = tails.tile([K, D], f32, tag="t_vf")
        nc.vector.tensor_scalar_mul(vf[:], vn[:], gb[:])
        nc.sync.dma_start(out=out_ap[n], in_=vf[:])


def kernel(x, conv_w, centroids):
    from concourse.bass_utils import run_bass_kernel_spmd

    if "nc" not in _CACHE:
        _CACHE["nc"] = _build_nc()
    nc = _CACHE["nc"]

    x = np.ascontiguousarray(np.asarray(x, dtype=np.float32).reshape(N, D, HW))
    wt = np.ascontiguousarray(np.asarray(conv_w, dtype=np.float32).T)
    cent = np.ascontiguousarray(np.asarray(centroids, dtype=np.float32))
    in_maps = [
        {"x": x[i * NS : (i + 1) * NS], "wt": wt, "cent": cent} for i in range(NCORES)
    ]
    res = run_bass_kernel_spmd(nc, in_maps, core_ids=list(range(NCORES))).results
    out = np.concatenate([r["out"].reshape(NS, K * D) for r in res], axis=0)
    return out


if __name__ == "__main__":
    rng = np.random.default_rng(0)
    xs = rng.standard_normal((N, D, 60, 80), dtype=np.float32)
    cw = (rng.standard_normal((K, D)) * 0.1).astype(np.float32)
    ct = rng.random((K, D), dtype=np.float32)
    o = kernel(x=xs, conv_w=cw, centroids=ct)
    print("kernel out", o.shape, o.dtype, np.abs(o).max())


# revision 22
# speedup vs baseline: 2.3252x; 1.3691x over previous
"""NetVLAD Trainium2 Bass kernel.

Full inputs in, full output out. Data-parallel over batch N=64 across 8
NeuronCores (8 samples per core); conv weight and centroids replicated.

Per-sample algorithm (mathematically equal to the reference, never
materializing the channel-normalized x):
  X = x[n]  [D=128, P=4800]  (D on SBUF partitions, contiguous in HBM)
  For each 128-wide chunk of P (p on partitions after a PE transpose):
    ss[p]    = sum_d X[d,p]^2
    inv_s    = ss^-0.5                   (DVE pow — keeps the ACT table
                                          set fixed: only Copy/Square/Exp)
    logitsT  = X_c^T @ Wt                (PE)
    e        = exp(logitsT * inv_s)      (softmax max-subtraction skipped:
                                          |logits*inv_s| <= ~1.2)
    sb       = e * (inv_s / Z),  Z = sum_k e
    acc     += [sb | e]^T @ [X_c^T | 1/Z]   (PE, PSUM accumulate)
  agg      = acc[0:64, 0:128];  sum_sa = acc[64:128, 128]
  vlad     = agg - sum_sa * centroids, then intra + global L2 norm.

Pipelining: per-chunk scalar work is batched into whole-sample ops
(one Square, one reduce, one Exp, ...); the accumulate matmuls of
sample n-2 are emitted between pass A of sample n so the PE never
waits on the scalar chain. The [sb|e] and [XT|1/Z] operands are bf16
(FWL fast weight load; f32 PSUM accumulation).
"""

import sys

if "/opt/trn_rl_repo" not in sys.path:
    sys.path.insert(0, "/opt/trn_rl_repo")

import numpy as np
from contextlib import ExitStack

N, D, HW, K = 64, 128, 4800, 64
NCORES = 8
NS = N // NCORES  # samples per core

CHUNKS = [(i * 128, min(128, HW - i * 128)) for i in range((HW + 127) // 128)]
NCH = len(CHUNKS)  # 38: 37 full + one 64-wide

_CACHE = {}


def _patch_act_tables():
    """Steer bacc's ACT table-set placement to the one set that covers
    every function we use (ln/exp/square/copy) so the kernel pays a single
    ACT_TABLE_LOAD instead of thrashing between per-anchor sets."""
    if _CACHE.get("act_patched"):
        return
    from concourse import bacc, mybir

    orig = bacc.get_activation_tables
    AF = mybir.ActivationFunctionType
    combo = "natural_log_exp_and_others"

    def patched(arch):
        t = {k: set(v) for k, v in orig(arch).items()}
        if combo in t:
            for name in t:
                if name != combo:
                    t[name] = t[name] - {AF.Ln, AF.Exp}
        return t

    bacc.get_activation_tables = patched
    _CACHE["act_patched"] = True


def _build_nc():
    import concourse.tile as tile
    from concourse import bacc, mybir

    _patch_act_tables()

    nc = bacc.Bacc(
        "TRN2",
        target_bir_lowering=False,
        debug=False,
        enable_asserts=False,
        num_devices=NCORES,
    )
    x_ap = nc.dram_tensor("x", [NS, D, HW], mybir.dt.float32, kind="ExternalInput").ap()
    wt_ap = nc.dram_tensor("wt", [D, K], mybir.dt.float32, kind="ExternalInput").ap()
    cent_ap = nc.dram_tensor(
        "cent", [K, D], mybir.dt.float32, kind="ExternalInput"
    ).ap()
    out_ap = nc.dram_tensor(
        "out", [NS, K, D], mybir.dt.float32, kind="ExternalOutput"
    ).ap()

    with tile.TileContext(nc) as tc:
        with ExitStack() as ctx:
            _body(ctx, tc, out_ap, x_ap, wt_ap, cent_ap)
    nc.compile()
    return nc


def _body(ctx, tc, out_ap, x_ap, wt_ap, cent_ap):
    import concourse.bass as bass
    from concourse import masks, mybir

    nc = tc.nc
    f32 = mybir.dt.float32
    bf16 = mybir.dt.bfloat16
    AF = mybir.ActivationFunctionType
    ALU = mybir.AluOpType
    X_AX = mybir.AxisListType.X

    singles = ctx.enter_context(tc.tile_pool(name="singles", bufs=1))
    xpool = ctx.enter_context(tc.tile_pool(name="xpool", bufs=2))
    xtrpool = ctx.enter_context(tc.tile_pool(name="xtrpool", bufs=3))
    ebpool = ctx.enter_context(tc.tile_pool(name="ebpool", bufs=3))
    lpool = ctx.enter_context(tc.tile_pool(name="lpool", bufs=2))
    scrpool = ctx.enter_context(tc.tile_pool(name="scrpool", bufs=2))
    smalls = ctx.enter_context(tc.tile_pool(name="smalls", bufs=3))
    tails = ctx.enter_context(tc.tile_pool(name="tails", bufs=1))
    pp_xt = ctx.enter_context(tc.tile_pool(name="pp_xt", bufs=2, space="PSUM"))
    pp_lg = ctx.enter_context(tc.tile_pool(name="pp_lg", bufs=2, space="PSUM"))
    pp_acc = ctx.enter_context(tc.tile_pool(name="pp_acc", bufs=2, space="PSUM"))
    pp_tiny = ctx.enter_context(tc.tile_pool(name="pp_tiny", bufs=1, space="PSUM"))

    def bcast(ap, n):
        # append a step-0 free dim: [..., n] broadcast view
        return bass.AP(tensor=ap.tensor, offset=ap.offset, ap=list(ap.ap) + [[0, n]])

    def mid_bcast(ap, n):
        # [p, f] -> [p, n, f] with step-0 middle dim
        return bass.AP(
            tensor=ap.tensor,
            offset=ap.offset,
            ap=[ap.ap[0], [0, n]] + list(ap.ap[1:]),
        )

    # constants
    ident = singles.tile([128, 128], f32)
    masks.make_identity(nc, ident[:])
    wt_s = singles.tile([D, K], f32)
    nc.sync.dma_start(out=wt_s[:], in_=wt_ap[:])
    cent_s = singles.tile([K, D], f32)
    nc.sync.dma_start(out=cent_s[:], in_=cent_ap[:])
    ones_col = singles.tile([K, 1], f32)
    nc.vector.memset(ones_col[:], 1.0)
    ones_row = singles.tile([1, K], f32)
    nc.vector.memset(ones_row[:], 1.0)

    GRP = 4  # transpose chunks per PSUM bank group
    groups = []
    c0 = 0
    while c0 < NCH:
        groups.append(list(range(c0, min(c0 + GRP, NCH))))
        c0 += GRP

    state = {}  # per-sample live tiles

    def emit_load_and_passA(n):
        xs = xpool.tile([D, HW], f32, tag="xs")
        nc.sync.dma_start(out=xs[:], in_=x_ap[n])

        # [XT | r] per chunk (bf16): cols 0:128 = X_c^T, col 128 = 1/Z
        xtr = xtrpool.tile([128, NCH, 129], bf16, tag="xtr")
        # [sb | e] per chunk (bf16)
        eb = ebpool.tile([128, NCH, 128], bf16, tag="eb")
        # raw logits stash (f32)
        lgs = lpool.tile([128, NCH, K], f32, tag="lgs")
        # XT^2 scratch (bf16 — ss reduce gets the 2x DVE mode)
        x2t = scrpool.tile([128, NCH * 128], bf16, tag="x2t")
        # scaled-logits scratch (f32)
        slgt = scrpool.tile([128, NCH * K], f32, tag="slgt")

        for grp in groups:
            gn = len(grp)
            xt_p = pp_xt.tile([128, GRP * 128], f32, tag="xt")
            lg_p = pp_lg.tile([128, GRP * K], f32, tag="lg")
            for j, c in enumerate(grp):
                p0, w = CHUNKS[c]
                x_c = xs[:, p0 : p0 + w]
                nc.tensor.matmul(
                    xt_p[:w, j * 128 : j * 128 + 128],
                    lhsT=x_c,
                    rhs=ident[:],
                    start=True,
                    stop=True,
                )
                nc.tensor.matmul(
                    lg_p[:w, j * K : (j + 1) * K],
                    lhsT=x_c,
                    rhs=wt_s[:],
                    start=True,
                    stop=True,
                )
            gc = grp[0]
            src_xt = xt_p[:, 0 : gn * 128].rearrange("p (c d) -> p c d", c=gn)
            nc.vector.tensor_copy(xtr[:, gc : gc + gn, 0:128], src_xt)
            src_lg = lg_p[:, 0 : gn * K].rearrange("p (c k) -> p c k", c=gn)
            nc.scalar.copy(lgs[:, gc : gc + gn, :], src_lg)

        state[n] = (xs, xtr, eb, lgs, x2t, slgt)

    def emit_scalars(n):
        xs, xtr, eb, lgs, x2t, slgt = state[n]
        ss = smalls.tile([128, NCH], f32, tag="ss")
        zz = smalls.tile([128, NCH], f32, tag="zz")
        is_ = smalls.tile([128, NCH], f32, tag="is")
        tsc = smalls.tile([128, NCH], bf16, tag="tsc")

        x2v = x2t[:].rearrange("p (c d) -> p c d", c=NCH)
        nc.scalar.activation(x2v, xtr[:, :, 0:128], AF.Square)
        nc.vector.tensor_reduce(out=ss[:], in_=x2v, axis=X_AX, op=ALU.add)
        # inv_s = exp(-0.5*ln(ss)); Ln+Exp live in one ACT table set
        lns = smalls.tile([128, NCH], f32, tag="lns")
        nc.scalar.activation(lns[:], ss[:], AF.Ln)
        nc.scalar.activation(is_[:], lns[:], AF.Exp, scale=-0.5)

        # scaled logits (f32) -> exp -> e (bf16)
        slg = slgt[:].rearrange("p (c k) -> p c k", c=NCH)
        nc.gpsimd.tensor_tensor(out=slg, in0=lgs[:], in1=bcast(is_[:], K), op=ALU.mult)
        nc.scalar.activation(eb[:, :, K : 2 * K], slg, AF.Exp)
        nc.vector.tensor_reduce(
            out=zz[:], in_=eb[:, :, K : 2 * K], axis=X_AX, op=ALU.add
        )
        # r = 1/Z into col 128 of each xtr chunk; t = inv_s * r (bf16)
        with nc.allow_low_precision(reason="r feeds the bf16 agg matmul"):
            nc.vector.reciprocal(xtr[:, :, 128], zz[:])
        nc.vector.tensor_tensor(out=tsc[:], in0=is_[:], in1=xtr[:, :, 128], op=ALU.mult)
        # sb = e * t -> eb[:,:,0:64]
        nc.gpsimd.tensor_tensor(
            out=eb[:, :, 0:K],
            in0=eb[:, :, K : 2 * K],
            in1=bcast(tsc[:], K),
            op=ALU.mult,
        )

    def emit_passC(n, agg_all, ssa_all):
        xs, xtr, eb, lgs, x2t, slgt = state.pop(n)
        acc_p = pp_acc.tile([128, 129], f32, tag="acc")
        for c, (p0, w) in enumerate(CHUNKS):
            nc.tensor.matmul(
                acc_p[:, :],
                lhsT=eb[:w, c, :],
                rhs=xtr[:w, c, :],
                start=(c == 0),
                stop=(c == NCH - 1),
            )
        # evacuate: agg rows 0:64 cols 0:128; sum_sa at [64:128, 128]
        nc.vector.tensor_copy(agg_all[:, n, :], acc_p[0:K, 0:D])
        nc.scalar.copy(ssa_all[:, n : n + 1], acc_p[K : 2 * K, 128:129])

    # batched across all samples
    agg_all = tails.tile([K, NS, D], f32)
    ssa_all = tails.tile([K, NS], f32)

    PIPE = 2  # pass C lags pass A by this many samples
    for n in range(NS):
        emit_load_and_passA(n)
        emit_scalars(n)
        if n >= PIPE:
            emit_passC(n - PIPE, agg_all, ssa_all)
    for n in range(NS - PIPE, NS):
        emit_passC(n, agg_all, ssa_all)

    # ---- batched tail over all samples ----
    vl = tails.tile([K, NS, D], f32)
    vsq = tails.tile([K, NS * D], f32)
    q = tails.tile([K, NS], f32)
    qm = tails.tile([K, NS], f32)
    isq = tails.tile([K, NS], f32)
    isq2 = tails.tile([K, NS], f32)
    u = tails.tile([K, NS], f32)
    gisr = tails.tile([1, NS], f32)
    gb = tails.tile([K, NS], f32)
    sall = tails.tile([K, NS], f32)
    vf = tails.tile([K, NS, D], f32)

    # vl = agg - ssa * cent
    nc.gpsimd.tensor_tensor(
        out=vl[:], in0=bcast(ssa_all[:], D), in1=mid_bcast(cent_s[:], NS), op=ALU.mult
    )
    nc.vector.tensor_tensor(out=vl[:], in0=agg_all[:], in1=vl[:], op=ALU.subtract)
    # q = rowsum(vl^2) per (k, n)
    vsqv = vsq[:].rearrange("k (n d) -> k n d", n=NS)
    nc.scalar.activation(vsqv, vl[:], AF.Square)
    nc.vector.tensor_reduce(out=q[:], in_=vsqv, axis=X_AX, op=ALU.add)
    nc.vector.tensor_scalar_max(qm[:], q[:], 1e-24)
    lq = tails.tile([K, NS], f32)
    nc.scalar.activation(lq[:], qm[:], AF.Ln)
    nc.scalar.activation(isq[:], lq[:], AF.Exp, scale=-0.5)
    # g = sum_k q_k * isq_k^2  (per sample)
    nc.vector.tensor_tensor(out=isq2[:], in0=isq[:], in1=isq[:], op=ALU.mult)
    nc.vector.tensor_tensor(out=u[:], in0=q[:], in1=isq2[:], op=ALU.mult)
    g_p = pp_tiny.tile([NS, 1], f32, tag="tiny")
    nc.tensor.matmul(g_p[:], lhsT=u[:], rhs=ones_col[:], start=True, stop=True)
    # gis = g^-0.5 -> transpose to a row -> broadcast over k partitions
    gm = tails.tile([NS, 1], f32)
    nc.vector.tensor_scalar_max(gm[:], g_p[:], 1e-24)
    gis = tails.tile([NS, 1], f32)
    lgm = tails.tile([NS, 1], f32)
    nc.scalar.activation(lgm[:], gm[:], AF.Ln)
    nc.scalar.activation(gis[:], lgm[:], AF.Exp, scale=-0.5)
    gr_p = pp_tiny.tile([1, NS], f32, tag="tiny")
    nc.tensor.matmul(gr_p[:], lhsT=gis[:], rhs=ident[:NS, :NS], start=True, stop=True)
    nc.vector.tensor_copy(gisr[:], gr_p[:])
    gb_p = pp_tiny.tile([K, NS], f32, tag="tiny")
    nc.tensor.matmul(gb_p[:], lhsT=ones_row[:], rhs=gisr[:], start=True, stop=True)
    nc.vector.tensor_copy(gb[:], gb_p[:])
    # s = isq * gb; vf = vl * s
    nc.vector.tensor_tensor(out=sall[:], in0=isq[:], in1=gb[:], op=ALU.mult)
    nc.gpsimd.tensor_tensor(out=vf[:], in0=vl[:], in1=bcast(sall[:], D), op=ALU.mult)
    nc.sync.dma_start(out=out_ap.rearrange("n k d -> k n d"), in_=vf[:])


def kernel(x, conv_w, centroids):
    from concourse.bass_utils import run_bass_kernel_spmd

    if "nc" not in _CACHE:
        _CACHE["nc"] = _build_nc()
    nc = _CACHE["nc"]

    x = np.ascontiguousarray(np.asarray(x, dtype=np.float32).reshape(N, D, HW))
    wt = np.ascontiguousarray(np.asarray(conv_w, dtype=np.float32).T)
    cent = np.ascontiguousarray(np.asarray(centroids, dtype=np.float32))
    in_maps = [
        {"x": x[i * NS : (i + 1) * NS], "wt": wt, "cent": cent} for i in range(NCORES)
    ]
    res = run_bass_kernel_spmd(nc, in_maps, core_ids=list(range(NCORES))).results
    out = np.concatenate([r["out"].reshape(NS, K * D) for r in res], axis=0)
    return out


if __name__ == "__main__":
    rng = np.random.default_rng(0)
    xs = rng.standard_normal((N, D, 60, 80), dtype=np.float32)
    cw = (rng.standard_normal((K, D)) * 0.1).astype(np.float32)
    ct = rng.random((K, D), dtype=np.float32)
    o = kernel(x=xs, conv_w=cw, centroids=ct)
    print("kernel out", o.shape, o.dtype, np.abs(o).max())


# revision 28
# speedup vs baseline: 2.4182x; 1.0400x over previous
"""NetVLAD Trainium2 Bass kernel.

Full inputs in, full output out. Data-parallel over batch N=64 across 8
NeuronCores (8 samples per core); conv weight and centroids replicated.

Per-sample algorithm (mathematically equal to the reference, never
materializing the channel-normalized x):
  X = x[n]  [D=128, P=4800]  (D on SBUF partitions, contiguous in HBM)
  For each 128-wide chunk of P (p on partitions after a PE transpose):
    ss[p]    = sum_d X[d,p]^2
    inv_s    = ss^-0.5                   (DVE pow — keeps the ACT table
                                          set fixed: only Copy/Square/Exp)
    logitsT  = X_c^T @ Wt                (PE)
    e        = exp(logitsT * inv_s)      (softmax max-subtraction skipped:
                                          |logits*inv_s| <= ~1.2)
    sb       = e * (inv_s / Z),  Z = sum_k e
    acc     += [sb | e]^T @ [X_c^T | 1/Z]   (PE, PSUM accumulate)
  agg      = acc[0:64, 0:128];  sum_sa = acc[64:128, 128]
  vlad     = agg - sum_sa * centroids, then intra + global L2 norm.

Pipelining: per-chunk scalar work is batched into whole-sample ops
(one Square, one reduce, one Exp, ...); the accumulate matmuls of
sample n-2 are emitted between pass A of sample n so the PE never
waits on the scalar chain. The [sb|e] and [XT|1/Z] operands are bf16
(FWL fast weight load; f32 PSUM accumulation).
"""

import sys

if "/opt/trn_rl_repo" not in sys.path:
    sys.path.insert(0, "/opt/trn_rl_repo")

import numpy as np
from contextlib import ExitStack

N, D, HW, K = 64, 128, 4800, 64
NCORES = 8
NS = N // NCORES  # samples per core

CHUNKS = [(i * 128, min(128, HW - i * 128)) for i in range((HW + 127) // 128)]
NCH = len(CHUNKS)  # 38: 37 full + one 64-wide

_CACHE = {}


def _patch_act_tables():
    """Steer bacc's ACT table-set placement to the one set that covers
    every function we use (ln/exp/square/copy) so the kernel pays a single
    ACT_TABLE_LOAD instead of thrashing between per-anchor sets."""
    if _CACHE.get("act_patched"):
        return
    from concourse import bacc, mybir

    orig = bacc.get_activation_tables
    AF = mybir.ActivationFunctionType
    combo = "natural_log_exp_and_others"

    def patched(arch):
        t = {k: set(v) for k, v in orig(arch).items()}
        if combo in t:
            for name in t:
                if name != combo:
                    t[name] = t[name] - {AF.Ln, AF.Exp}
        return t

    bacc.get_activation_tables = patched
    _CACHE["act_patched"] = True


def _build_nc():
    import concourse.tile as tile
    from concourse import bacc, mybir

    _patch_act_tables()

    nc = bacc.Bacc(
        "TRN2",
        target_bir_lowering=False,
        debug=False,
        enable_asserts=False,
        num_devices=NCORES,
    )
    x_ap = nc.dram_tensor("x", [NS, D, HW], mybir.dt.float32, kind="ExternalInput").ap()
    wt_ap = nc.dram_tensor("wt", [D, K], mybir.dt.float32, kind="ExternalInput").ap()
    cent_ap = nc.dram_tensor(
        "cent", [K, D], mybir.dt.float32, kind="ExternalInput"
    ).ap()
    out_ap = nc.dram_tensor(
        "out", [NS, K, D], mybir.dt.float32, kind="ExternalOutput"
    ).ap()

    with tile.TileContext(nc) as tc:
        with ExitStack() as ctx:
            _body(ctx, tc, out_ap, x_ap, wt_ap, cent_ap)
    nc.compile()
    return nc


def _body(ctx, tc, out_ap, x_ap, wt_ap, cent_ap):
    import concourse.bass as bass
    from concourse import masks, mybir

    nc = tc.nc
    f32 = mybir.dt.float32
    bf16 = mybir.dt.bfloat16
    AF = mybir.ActivationFunctionType
    ALU = mybir.AluOpType
    X_AX = mybir.AxisListType.X

    singles = ctx.enter_context(tc.tile_pool(name="singles", bufs=1))
    xpool = ctx.enter_context(tc.tile_pool(name="xpool", bufs=2))
    xtrpool = ctx.enter_context(tc.tile_pool(name="xtrpool", bufs=4))
    ebpool = ctx.enter_context(tc.tile_pool(name="ebpool", bufs=2))
    sbtpool = ctx.enter_context(tc.tile_pool(name="sbtpool", bufs=4))
    lpool = ctx.enter_context(tc.tile_pool(name="lpool", bufs=2))
    scrpool = ctx.enter_context(tc.tile_pool(name="scrpool", bufs=2))
    smalls = ctx.enter_context(tc.tile_pool(name="smalls", bufs=3))
    tails = ctx.enter_context(tc.tile_pool(name="tails", bufs=1))
    pp_xt = ctx.enter_context(tc.tile_pool(name="pp_xt", bufs=2, space="PSUM"))
    pp_acc = ctx.enter_context(tc.tile_pool(name="pp_acc", bufs=2, space="PSUM"))
    pp_tiny = ctx.enter_context(tc.tile_pool(name="pp_tiny", bufs=1, space="PSUM"))

    def bcast(ap, n):
        # append a step-0 free dim: [..., n] broadcast view
        return bass.AP(tensor=ap.tensor, offset=ap.offset, ap=list(ap.ap) + [[0, n]])

    def mid_bcast(ap, n):
        # [p, f] -> [p, n, f] with step-0 middle dim
        return bass.AP(
            tensor=ap.tensor,
            offset=ap.offset,
            ap=[ap.ap[0], [0, n]] + list(ap.ap[1:]),
        )

    # constants
    ident = singles.tile([128, 128], f32)
    masks.make_identity(nc, ident[:])
    # fused rhs for pass A: [identity | Wt] — one matmul yields [X_c^T | logits]
    identwt = singles.tile([128, 192], f32)
    masks.make_identity(nc, identwt[:, 0:128])
    nc.sync.dma_start(out=identwt[:, 128:192], in_=wt_ap[:])
    wt_s = singles.tile([D, K], f32)
    nc.sync.dma_start(out=wt_s[:], in_=wt_ap[:])
    cent_s = singles.tile([K, D], f32)
    nc.sync.dma_start(out=cent_s[:], in_=cent_ap[:])
    ones_col = singles.tile([K, 1], f32)
    nc.vector.memset(ones_col[:], 1.0)
    ones_row = singles.tile([1, K], f32)
    nc.vector.memset(ones_row[:], 1.0)

    GRP = 4  # transpose chunks per PSUM bank group
    groups = []
    c0 = 0
    while c0 < NCH:
        groups.append(list(range(c0, min(c0 + GRP, NCH))))
        c0 += GRP

    state = {}  # per-sample live tiles

    def emit_load_and_passA(n):
        xs = xpool.tile([D, HW], f32, tag="xs")
        nc.sync.dma_start(out=xs[:], in_=x_ap[n])

        # [XT | s] per chunk (bf16): cols 0:128 = X_c^T, col 128 = ||x_p||
        xtr = xtrpool.tile([128, NCH, 129], bf16, tag="xtr")
        # softmax numerators e (bf16, contiguous for the 2x reduce)
        et = ebpool.tile([128, NCH, K], bf16, tag="et")
        # sb = e * inv_s/Z — the acc matmul's stationary operand
        sbt = sbtpool.tile([128, NCH, K], bf16, tag="sbt")
        # raw logits stash (f32)
        lgs = lpool.tile([128, NCH, K], f32, tag="lgs")
        # XT^2 scratch (bf16 — ss reduce gets the 2x DVE mode)
        x2t = scrpool.tile([128, NCH * 128], bf16, tag="x2t")
        # scaled-logits scratch (f32)
        slgt = scrpool.tile([128, NCH * K], f32, tag="slgt")

        for grp in groups:
            gn = len(grp)
            # one fused matmul per chunk: out cols 0:128 = X_c^T, 128:192 =
            # logits. 256-col stride keeps each 192-col output in one bank.
            xt_p = pp_xt.tile([128, GRP, 256], f32, tag="xt")
            for j, c in enumerate(grp):
                p0, w = CHUNKS[c]
                x_c = xs[:, p0 : p0 + w]
                nc.tensor.matmul(
                    xt_p[:w, j, 0:192],
                    lhsT=x_c,
                    rhs=identwt[:],
                    start=True,
                    stop=True,
                )
            gc = grp[0]
            nc.vector.tensor_copy(xtr[:, gc : gc + gn, 0:128], xt_p[:, 0:gn, 0:128])
            nc.scalar.copy(lgs[:, gc : gc + gn, :], xt_p[:, 0:gn, 128:192])

        state[n] = (xs, xtr, et, sbt, lgs, x2t, slgt)

    def emit_scalars(n):
        xs, xtr, et, sbt, lgs, x2t, slgt = state[n]
        ss = smalls.tile([128, NCH], f32, tag="ss")
        zz = smalls.tile([128, NCH], f32, tag="zz")
        is_ = smalls.tile([128, NCH], f32, tag="is")
        tsc = smalls.tile([128, NCH], bf16, tag="tsc")

        x2v = x2t[:].rearrange("p (c d) -> p c d", c=NCH)
        nc.scalar.activation(x2v, xtr[:, :, 0:128], AF.Square)
        nc.vector.tensor_reduce(out=ss[:], in_=x2v, axis=X_AX, op=ALU.add)
        # inv_s = exp(-0.5*ln(ss)); Ln+Exp live in one ACT table set
        lns = smalls.tile([128, NCH], f32, tag="lns")
        nc.scalar.activation(lns[:], ss[:], AF.Ln)
        nc.scalar.activation(is_[:], lns[:], AF.Exp, scale=-0.5)
        # s = ss * inv_s = ||x_p||, into col 128 of each xtr chunk (the acc
        # matmul's rhs column turning sb into sum_sa)
        with nc.allow_low_precision(reason="s feeds the bf16 agg matmul"):
            nc.vector.tensor_tensor(
                out=xtr[:, :, 128], in0=ss[:], in1=is_[:], op=ALU.mult
            )

        # split the rest per half-sample to shorten the dependency chain
        halves = [(0, NCH // 2), (NCH // 2, NCH)]
        for h0, h1 in halves:
            slg = slgt[:, h0 * K : h1 * K].rearrange("p (c k) -> p c k", c=h1 - h0)
            nc.gpsimd.tensor_tensor(
                out=slg,
                in0=lgs[:, h0:h1, :],
                in1=bcast(is_[:, h0:h1], K),
                op=ALU.mult,
            )
            nc.scalar.activation(et[:, h0:h1, :], slg, AF.Exp)
            nc.vector.tensor_reduce(
                out=zz[:, h0:h1], in_=et[:, h0:h1, :], axis=X_AX, op=ALU.add
            )
            rr = smalls.tile([128, NCH // 2], f32, tag="rr")
            nc.vector.reciprocal(rr[:], zz[:, h0:h1])
            # t = inv_s / Z
            nc.vector.tensor_tensor(
                out=tsc[:, h0:h1], in0=is_[:, h0:h1], in1=rr[:], op=ALU.mult
            )
            # sb = e * t
            nc.gpsimd.tensor_tensor(
                out=sbt[:, h0:h1, :],
                in0=et[:, h0:h1, :],
                in1=bcast(tsc[:, h0:h1], K),
                op=ALU.mult,
            )

    def emit_passC(n, agg_all, ssa_all):
        xs, xtr, et, sbt, lgs, x2t, slgt = state.pop(n)
        acc_p = pp_acc.tile([K, 129], f32, tag="acc")
        for c, (p0, w) in enumerate(CHUNKS):
            nc.tensor.matmul(
                acc_p[:, :],
                lhsT=sbt[:w, c, :],
                rhs=xtr[:w, c, :],
                start=(c == 0),
                stop=(c == NCH - 1),
            )
        # evacuate: agg = cols 0:128; sum_sa = col 128
        nc.vector.tensor_copy(agg_all[:, n, :], acc_p[:, 0:D])
        nc.scalar.copy(ssa_all[:, n : n + 1], acc_p[:, 128:129])

    # batched across all samples
    agg_all = tails.tile([K, NS, D], f32)
    ssa_all = tails.tile([K, NS], f32)

    PIPE = 3  # pass C lags pass A by this many samples
    for n in range(NS):
        emit_load_and_passA(n)
        emit_scalars(n)
        if n >= PIPE:
            emit_passC(n - PIPE, agg_all, ssa_all)
    for n in range(NS - PIPE, NS):
        emit_passC(n, agg_all, ssa_all)

    # ---- batched tail over all samples ----
    vl = tails.tile([K, NS, D], f32)
    vsq = tails.tile([K, NS * D], f32)
    q = tails.tile([K, NS], f32)
    qm = tails.tile([K, NS], f32)
    isq = tails.tile([K, NS], f32)
    isq2 = tails.tile([K, NS], f32)
    u = tails.tile([K, NS], f32)
    gisr = tails.tile([1, NS], f32)
    gb = tails.tile([K, NS], f32)
    sall = tails.tile([K, NS], f32)
    vf = tails.tile([K, NS, D], f32)

    # vl = agg - ssa * cent
    nc.gpsimd.tensor_tensor(
        out=vl[:], in0=bcast(ssa_all[:], D), in1=mid_bcast(cent_s[:], NS), op=ALU.mult
    )
    nc.vector.tensor_tensor(out=vl[:], in0=agg_all[:], in1=vl[:], op=ALU.subtract)
    # q = rowsum(vl^2) per (k, n)
    vsqv = vsq[:].rearrange("k (n d) -> k n d", n=NS)
    nc.scalar.activation(vsqv, vl[:], AF.Square)
    nc.vector.tensor_reduce(out=q[:], in_=vsqv, axis=X_AX, op=ALU.add)
    nc.vector.tensor_scalar_max(qm[:], q[:], 1e-24)
    lq = tails.tile([K, NS], f32)
    nc.scalar.activation(lq[:], qm[:], AF.Ln)
    nc.scalar.activation(isq[:], lq[:], AF.Exp, scale=-0.5)
    # g = sum_k q_k * isq_k^2  (per sample)
    nc.vector.tensor_tensor(out=isq2[:], in0=isq[:], in1=isq[:], op=ALU.mult)
    nc.vector.tensor_tensor(out=u[:], in0=q[:], in1=isq2[:], op=ALU.mult)
    g_p = pp_tiny.tile([NS, 1], f32, tag="tiny")
    nc.tensor.matmul(g_p[:], lhsT=u[:], rhs=ones_col[:], start=True, stop=True)
    # gis = g^-0.5 -> transpose to a row -> broadcast over k partitions
    gm = tails.tile([NS, 1], f32)
    nc.vector.tensor_scalar_max(gm[:], g_p[:], 1e-24)
    gis = tails.tile([NS, 1], f32)
    lgm = tails.tile([NS, 1], f32)
    nc.scalar.activation(lgm[:], gm[:], AF.Ln)
    nc.scalar.activation(gis[:], lgm[:], AF.Exp, scale=-0.5)
    gr_p = pp_tiny.tile([1, NS], f32, tag="tiny")
    nc.tensor.matmul(gr_p[:], lhsT=gis[:], rhs=ident[:NS, :NS], start=True, stop=True)
    nc.vector.tensor_copy(gisr[:], gr_p[:])
    gb_p = pp_tiny.tile([K, NS], f32, tag="tiny")
    nc.tensor.matmul(gb_p[:], lhsT=ones_row[:], rhs=gisr[:], start=True, stop=True)
    nc.vector.tensor_copy(gb[:], gb_p[:])
    # s = isq * gb; vf = vl * s
    nc.vector.tensor_tensor(out=sall[:], in0=isq[:], in1=gb[:], op=ALU.mult)
    nc.gpsimd.tensor_tensor(out=vf[:], in0=vl[:], in1=bcast(sall[:], D), op=ALU.mult)
    nc.sync.dma_start(out=out_ap.rearrange("n k d -> k n d"), in_=vf[:])


def kernel(x, conv_w, centroids):
    from concourse.bass_utils import run_bass_kernel_spmd

    if "nc" not in _CACHE:
        _CACHE["nc"] = _build_nc()
    nc = _CACHE["nc"]

    x = np.ascontiguousarray(np.asarray(x, dtype=np.float32).reshape(N, D, HW))
    wt = np.ascontiguousarray(np.asarray(conv_w, dtype=np.float32).T)
    cent = np.ascontiguousarray(np.asarray(centroids, dtype=np.float32))
    in_maps = [
        {"x": x[i * NS : (i + 1) * NS], "wt": wt, "cent": cent} for i in range(NCORES)
    ]
    res = run_bass_kernel_spmd(nc, in_maps, core_ids=list(range(NCORES))).results
    out = np.concatenate([r["out"].reshape(NS, K * D) for r in res], axis=0)
    return out


if __name__ == "__main__":
    rng = np.random.default_rng(0)
    xs = rng.standard_normal((N, D, 60, 80), dtype=np.float32)
    cw = (rng.standard_normal((K, D)) * 0.1).astype(np.float32)
    ct = rng.random((K, D), dtype=np.float32)
    o = kernel(x=xs, conv_w=cw, centroids=ct)
    print("kernel out", o.shape, o.dtype, np.abs(o).max())


# revision 30
# speedup vs baseline: 2.8549x; 1.1806x over previous
"""NetVLAD Trainium2 Bass kernel.

Full inputs in, full output out. Data-parallel over batch N=64 across 8
NeuronCores (8 samples per core); conv weight and centroids replicated.

Per-sample algorithm (mathematically equal to the reference, never
materializing the channel-normalized x):
  X = x[n]  [D=128, P=4800]  (D on SBUF partitions, contiguous in HBM)
  For each 128-wide chunk of P (p on partitions after a PE transpose):
    ss[p]    = sum_d X[d,p]^2
    inv_s    = ss^-0.5                   (DVE pow — keeps the ACT table
                                          set fixed: only Copy/Square/Exp)
    logitsT  = X_c^T @ Wt                (PE)
    e        = exp(logitsT * inv_s)      (softmax max-subtraction skipped:
                                          |logits*inv_s| <= ~1.2)
    sb       = e * (inv_s / Z),  Z = sum_k e
    acc     += [sb | e]^T @ [X_c^T | 1/Z]   (PE, PSUM accumulate)
  agg      = acc[0:64, 0:128];  sum_sa = acc[64:128, 128]
  vlad     = agg - sum_sa * centroids, then intra + global L2 norm.

Pipelining: per-chunk scalar work is batched into whole-sample ops
(one Square, one reduce, one Exp, ...); the accumulate matmuls of
sample n-2 are emitted between pass A of sample n so the PE never
waits on the scalar chain. The [sb|e] and [XT|1/Z] operands are bf16
(FWL fast weight load; f32 PSUM accumulation).
"""

import sys

if "/opt/trn_rl_repo" not in sys.path:
    sys.path.insert(0, "/opt/trn_rl_repo")

import numpy as np
from contextlib import ExitStack

N, D, HW, K = 64, 128, 4800, 64
NCORES = 8
NS = N // NCORES  # samples per core

CHUNKS = [(i * 128, min(128, HW - i * 128)) for i in range((HW + 127) // 128)]
NCH = len(CHUNKS)  # 38: 37 full + one 64-wide

_CACHE = {}


def _patch_act_tables():
    """Steer bacc's ACT table-set placement to the one set that covers
    every function we use (ln/exp/square/copy) so the kernel pays a single
    ACT_TABLE_LOAD instead of thrashing between per-anchor sets."""
    if _CACHE.get("act_patched"):
        return
    from concourse import bacc, mybir

    orig = bacc.get_activation_tables
    AF = mybir.ActivationFunctionType
    combo = "natural_log_exp_and_others"

    def patched(arch):
        t = {k: set(v) for k, v in orig(arch).items()}
        if combo in t:
            for name in t:
                if name != combo:
                    t[name] = t[name] - {AF.Ln, AF.Exp}
        return t

    bacc.get_activation_tables = patched
    _CACHE["act_patched"] = True


def _build_nc():
    import concourse.tile as tile
    from concourse import bacc, mybir

    _patch_act_tables()

    nc = bacc.Bacc(
        "TRN2",
        target_bir_lowering=False,
        debug=False,
        enable_asserts=False,
        num_devices=NCORES,
    )
    x_ap = nc.dram_tensor("x", [NS, D, HW], mybir.dt.float32, kind="ExternalInput").ap()
    wt_ap = nc.dram_tensor("wt", [D, K], mybir.dt.float32, kind="ExternalInput").ap()
    cent_ap = nc.dram_tensor(
        "cent", [K, D], mybir.dt.float32, kind="ExternalInput"
    ).ap()
    out_ap = nc.dram_tensor(
        "out", [NS, K, D], mybir.dt.float32, kind="ExternalOutput"
    ).ap()

    with tile.TileContext(nc) as tc:
        with ExitStack() as ctx:
            _body(ctx, tc, out_ap, x_ap, wt_ap, cent_ap)
    nc.compile()
    return nc


def _body(ctx, tc, out_ap, x_ap, wt_ap, cent_ap):
    import concourse.bass as bass
    from concourse import masks, mybir

    nc = tc.nc
    f32 = mybir.dt.float32
    bf16 = mybir.dt.bfloat16
    AF = mybir.ActivationFunctionType
    ALU = mybir.AluOpType
    X_AX = mybir.AxisListType.X

    singles = ctx.enter_context(tc.tile_pool(name="singles", bufs=1))
    xpool = ctx.enter_context(tc.tile_pool(name="xpool", bufs=2))
    xtrpool = ctx.enter_context(tc.tile_pool(name="xtrpool", bufs=4))
    ebpool = ctx.enter_context(tc.tile_pool(name="ebpool", bufs=2))
    sbtpool = ctx.enter_context(tc.tile_pool(name="sbtpool", bufs=4))
    lpool = ctx.enter_context(tc.tile_pool(name="lpool", bufs=2))
    scrpool = ctx.enter_context(tc.tile_pool(name="scrpool", bufs=2))
    smalls = ctx.enter_context(tc.tile_pool(name="smalls", bufs=3))
    tails = ctx.enter_context(tc.tile_pool(name="tails", bufs=1))
    pp_xt = ctx.enter_context(tc.tile_pool(name="pp_xt", bufs=3, space="PSUM"))
    pp_acc = ctx.enter_context(tc.tile_pool(name="pp_acc", bufs=1, space="PSUM"))
    pp_tiny = ctx.enter_context(tc.tile_pool(name="pp_tiny", bufs=1, space="PSUM"))

    def bcast(ap, n):
        # append a step-0 free dim: [..., n] broadcast view
        return bass.AP(tensor=ap.tensor, offset=ap.offset, ap=list(ap.ap) + [[0, n]])

    def mid_bcast(ap, n):
        # [p, f] -> [p, n, f] with step-0 middle dim
        return bass.AP(
            tensor=ap.tensor,
            offset=ap.offset,
            ap=[ap.ap[0], [0, n]] + list(ap.ap[1:]),
        )

    # constants
    ident = singles.tile([128, 128], f32)
    masks.make_identity(nc, ident[:])
    # fused rhs for pass A: [identity | Wt] — one matmul yields [X_c^T | logits]
    identwt = singles.tile([128, 192], f32)
    masks.make_identity(nc, identwt[:, 0:128])
    nc.sync.dma_start(out=identwt[:, 128:192], in_=wt_ap[:])
    wt_s = singles.tile([D, K], f32)
    nc.sync.dma_start(out=wt_s[:], in_=wt_ap[:])
    cent_s = singles.tile([K, D], f32)
    nc.sync.dma_start(out=cent_s[:], in_=cent_ap[:])
    ones_col = singles.tile([K, 1], f32)
    nc.vector.memset(ones_col[:], 1.0)
    ones_row = singles.tile([1, K], f32)
    nc.vector.memset(ones_row[:], 1.0)

    GRP = 4  # transpose chunks per PSUM bank group
    groups = []
    c0 = 0
    while c0 < NCH:
        groups.append(list(range(c0, min(c0 + GRP, NCH))))
        c0 += GRP

    state = {}  # per-sample live tiles

    def emit_load_and_passA(n):
        xs = xpool.tile([D, HW], f32, tag="xs")
        nc.sync.dma_start(out=xs[:], in_=x_ap[n])

        # [XT | s] per chunk (bf16): cols 0:128 = X_c^T, col 128 = ||x_p||
        xtr = xtrpool.tile([128, NCH, 129], bf16, tag="xtr")
        # softmax numerators e (bf16, contiguous for the 2x reduce)
        et = ebpool.tile([128, NCH, K], bf16, tag="et")
        # sb = e * inv_s/Z — the acc matmul's stationary operand
        sbt = sbtpool.tile([128, NCH, K], bf16, tag="sbt")
        # raw logits stash (f32)
        lgs = lpool.tile([128, NCH, K], f32, tag="lgs")
        # XT^2 scratch (bf16 — ss reduce gets the 2x DVE mode)
        x2t = scrpool.tile([128, NCH * 128], bf16, tag="x2t")
        # scaled-logits scratch (f32)
        slgt = scrpool.tile([128, NCH * K], f32, tag="slgt")

        for grp in groups:
            gn = len(grp)
            # one fused matmul per chunk: out cols 0:128 = X_c^T, 128:192 =
            # logits. 256-col stride keeps each 192-col output in one bank.
            xt_p = pp_xt.tile([128, GRP, 256], f32, tag="xt")
            for j, c in enumerate(grp):
                p0, w = CHUNKS[c]
                x_c = xs[:, p0 : p0 + w]
                nc.tensor.matmul(
                    xt_p[:w, j, 0:192],
                    lhsT=x_c,
                    rhs=identwt[:],
                    start=True,
                    stop=True,
                )
            gc = grp[0]
            nc.vector.tensor_copy(xtr[:, gc : gc + gn, 0:128], xt_p[:, 0:gn, 0:128])
            nc.scalar.copy(lgs[:, gc : gc + gn, :], xt_p[:, 0:gn, 128:192])

        state[n] = (xs, xtr, et, sbt, lgs, x2t, slgt)

    def emit_scalars(n):
        xs, xtr, et, sbt, lgs, x2t, slgt = state[n]
        ss = smalls.tile([128, NCH], f32, tag="ss")
        zz = smalls.tile([128, NCH], f32, tag="zz")
        is_ = smalls.tile([128, NCH], f32, tag="is")
        tsc = smalls.tile([128, NCH], bf16, tag="tsc")

        # everything split per half-sample: shorter dependency links, and
        # the static per-engine schedule interleaves across samples better
        halves = [(0, NCH // 2), (NCH // 2, NCH)]
        lns = smalls.tile([128, NCH], f32, tag="lns")
        x2vf = x2t[:].rearrange("p (c d) -> p c d", c=NCH)
        for h0, h1 in halves:
            nc.scalar.activation(x2vf[:, h0:h1, :], xtr[:, h0:h1, 0:128], AF.Square)
            nc.vector.tensor_reduce(
                out=ss[:, h0:h1], in_=x2vf[:, h0:h1, :], axis=X_AX, op=ALU.add
            )
            # inv_s = exp(-0.5*ln(ss)); Ln+Exp live in one ACT table set
            nc.scalar.activation(lns[:, h0:h1], ss[:, h0:h1], AF.Ln)
            nc.scalar.activation(is_[:, h0:h1], lns[:, h0:h1], AF.Exp, scale=-0.5)
            # s = ss * inv_s = ||x_p||, into col 128 of each xtr chunk (the
            # acc matmul's rhs column turning sb into sum_sa)
            with nc.allow_low_precision(reason="s feeds the bf16 agg matmul"):
                nc.vector.tensor_tensor(
                    out=xtr[:, h0:h1, 128],
                    in0=ss[:, h0:h1],
                    in1=is_[:, h0:h1],
                    op=ALU.mult,
                )

        for h0, h1 in halves:
            slg = slgt[:, h0 * K : h1 * K].rearrange("p (c k) -> p c k", c=h1 - h0)
            nc.gpsimd.tensor_tensor(
                out=slg,
                in0=lgs[:, h0:h1, :],
                in1=bcast(is_[:, h0:h1], K),
                op=ALU.mult,
            )
            nc.scalar.activation(et[:, h0:h1, :], slg, AF.Exp)
            nc.vector.tensor_reduce(
                out=zz[:, h0:h1], in_=et[:, h0:h1, :], axis=X_AX, op=ALU.add
            )
            rr = smalls.tile([128, NCH // 2], f32, tag="rr")
            nc.vector.reciprocal(rr[:], zz[:, h0:h1])
            # t = inv_s / Z
            nc.vector.tensor_tensor(
                out=tsc[:, h0:h1], in0=is_[:, h0:h1], in1=rr[:], op=ALU.mult
            )
            # sb = e * t
            nc.gpsimd.tensor_tensor(
                out=sbt[:, h0:h1, :],
                in0=et[:, h0:h1, :],
                in1=bcast(tsc[:, h0:h1], K),
                op=ALU.mult,
            )

    def emit_passC(n, agg_all, ssa_all):
        xs, xtr, et, sbt, lgs, x2t, slgt = state.pop(n)
        acc_p = pp_acc.tile([K, 129], f32, tag="acc")
        for c, (p0, w) in enumerate(CHUNKS):
            nc.tensor.matmul(
                acc_p[:, :],
                lhsT=sbt[:w, c, :],
                rhs=xtr[:w, c, :],
                start=(c == 0),
                stop=(c == NCH - 1),
            )
        # evacuate: agg = cols 0:128; sum_sa = col 128
        nc.vector.tensor_copy(agg_all[:, n, :], acc_p[:, 0:D])
        nc.scalar.copy(ssa_all[:, n : n + 1], acc_p[:, 128:129])

    # batched across all samples
    agg_all = tails.tile([K, NS, D], f32)
    ssa_all = tails.tile([K, NS], f32)

    PIPE = 3  # pass C lags pass A by this many samples
    for n in range(NS):
        emit_load_and_passA(n)
        emit_scalars(n)
        if n >= PIPE:
            emit_passC(n - PIPE, agg_all, ssa_all)
    for n in range(NS - PIPE, NS):
        emit_passC(n, agg_all, ssa_all)

    # ---- batched tail over all samples ----
    vl = tails.tile([K, NS, D], f32)
    vsq = tails.tile([K, NS * D], f32)
    q = tails.tile([K, NS], f32)
    qm = tails.tile([K, NS], f32)
    isq = tails.tile([K, NS], f32)
    isq2 = tails.tile([K, NS], f32)
    u = tails.tile([K, NS], f32)
    gisr = tails.tile([1, NS], f32)
    gb = tails.tile([K, NS], f32)
    sall = tails.tile([K, NS], f32)
    vf = tails.tile([K, NS, D], f32)

    # vl = agg - ssa * cent
    nc.gpsimd.tensor_tensor(
        out=vl[:], in0=bcast(ssa_all[:], D), in1=mid_bcast(cent_s[:], NS), op=ALU.mult
    )
    nc.vector.tensor_tensor(out=vl[:], in0=agg_all[:], in1=vl[:], op=ALU.subtract)
    # q = rowsum(vl^2) per (k, n)
    vsqv = vsq[:].rearrange("k (n d) -> k n d", n=NS)
    nc.scalar.activation(vsqv, vl[:], AF.Square)
    nc.vector.tensor_reduce(out=q[:], in_=vsqv, axis=X_AX, op=ALU.add)
    nc.vector.tensor_scalar_max(qm[:], q[:], 1e-24)
    lq = tails.tile([K, NS], f32)
    nc.scalar.activation(lq[:], qm[:], AF.Ln)
    nc.scalar.activation(isq[:], lq[:], AF.Exp, scale=-0.5)
    # g = sum_k q_k * isq_k^2  (per sample)
    nc.vector.tensor_tensor(out=isq2[:], in0=isq[:], in1=isq[:], op=ALU.mult)
    nc.vector.tensor_tensor(out=u[:], in0=q[:], in1=isq2[:], op=ALU.mult)
    g_p = pp_tiny.tile([NS, 1], f32, tag="tiny")
    nc.tensor.matmul(g_p[:], lhsT=u[:], rhs=ones_col[:], start=True, stop=True)
    # gis = g^-0.5 -> transpose to a row -> broadcast over k partitions
    gm = tails.tile([NS, 1], f32)
    nc.vector.tensor_scalar_max(gm[:], g_p[:], 1e-24)
    gis = tails.tile([NS, 1], f32)
    lgm = tails.tile([NS, 1], f32)
    nc.scalar.activation(lgm[:], gm[:], AF.Ln)
    nc.scalar.activation(gis[:], lgm[:], AF.Exp, scale=-0.5)
    gr_p = pp_tiny.tile([1, NS], f32, tag="tiny")
    nc.tensor.matmul(gr_p[:], lhsT=gis[:], rhs=ident[:NS, :NS], start=True, stop=True)
    nc.vector.tensor_copy(gisr[:], gr_p[:])
    gb_p = pp_tiny.tile([K, NS], f32, tag="tiny")
    nc.tensor.matmul(gb_p[:], lhsT=ones_row[:], rhs=gisr[:], start=True, stop=True)
    nc.vector.tensor_copy(gb[:], gb_p[:])
    # s = isq * gb; vf = vl * s
    nc.vector.tensor_tensor(out=sall[:], in0=isq[:], in1=gb[:], op=ALU.mult)
    nc.gpsimd.tensor_tensor(out=vf[:], in0=vl[:], in1=bcast(sall[:], D), op=ALU.mult)
    nc.sync.dma_start(out=out_ap.rearrange("n k d -> k n d"), in_=vf[:])


def kernel(x, conv_w, centroids):
    from concourse.bass_utils import run_bass_kernel_spmd

    if "nc" not in _CACHE:
        _CACHE["nc"] = _build_nc()
    nc = _CACHE["nc"]

    x = np.ascontiguousarray(np.asarray(x, dtype=np.float32).reshape(N, D, HW))
    wt = np.ascontiguousarray(np.asarray(conv_w, dtype=np.float32).T)
    cent = np.ascontiguousarray(np.asarray(centroids, dtype=np.float32))
    in_maps = [
        {"x": x[i * NS : (i + 1) * NS], "wt": wt, "cent": cent} for i in range(NCORES)
    ]
    res = run_bass_kernel_spmd(nc, in_maps, core_ids=list(range(NCORES))).results
    out = np.concatenate([r["out"].reshape(NS, K * D) for r in res], axis=0)
    return out


if __name__ == "__main__":
    rng = np.random.default_rng(0)
    xs = rng.standard_normal((N, D, 60, 80), dtype=np.float32)
    cw = (rng.standard_normal((K, D)) * 0.1).astype(np.float32)
    ct = rng.random((K, D), dtype=np.float32)
    o = kernel(x=xs, conv_w=cw, centroids=ct)
    print("kernel out", o.shape, o.dtype, np.abs(o).max())


# revision 36
# speedup vs baseline: 3.1350x; 1.0981x over previous
"""NetVLAD Trainium2 Bass kernel.

Full inputs in, full output out. Data-parallel over batch N=64 across 8
NeuronCores (8 samples per core); conv weight and centroids replicated.

Per-sample algorithm (mathematically equal to the reference, never
materializing the channel-normalized x):
  X = x[n]  [D=128, P=4800]  (D on SBUF partitions, contiguous in HBM)
  For each 128-wide chunk of P (p on partitions after a PE transpose):
    ss[p]    = sum_d X[d,p]^2
    inv_s    = ss^-0.5                   (DVE pow — keeps the ACT table
                                          set fixed: only Copy/Square/Exp)
    logitsT  = X_c^T @ Wt                (PE)
    e        = exp(logitsT * inv_s)      (softmax max-subtraction skipped:
                                          |logits*inv_s| <= ~1.2)
    sb       = e * (inv_s / Z),  Z = sum_k e
    acc     += [sb | e]^T @ [X_c^T | 1/Z]   (PE, PSUM accumulate)
  agg      = acc[0:64, 0:128];  sum_sa = acc[64:128, 128]
  vlad     = agg - sum_sa * centroids, then intra + global L2 norm.

Pipelining: per-chunk scalar work is batched into whole-sample ops
(one Square, one reduce, one Exp, ...); the accumulate matmuls of
sample n-2 are emitted between pass A of sample n so the PE never
waits on the scalar chain. The [sb|e] and [XT|1/Z] operands are bf16
(FWL fast weight load; f32 PSUM accumulation).
"""

import sys

if "/opt/trn_rl_repo" not in sys.path:
    sys.path.insert(0, "/opt/trn_rl_repo")

import numpy as np
from contextlib import ExitStack

N, D, HW, K = 64, 128, 4800, 64
NCORES = 8
NS = N // NCORES  # samples per core

CHUNKS = [(i * 128, min(128, HW - i * 128)) for i in range((HW + 127) // 128)]
NCH = len(CHUNKS)  # 38: 37 full + one 64-wide

_CACHE = {}


def _patch_act_tables():
    """Steer bacc's ACT table-set placement to the one set that covers
    every function we use (ln/exp/square/copy) so the kernel pays a single
    ACT_TABLE_LOAD instead of thrashing between per-anchor sets."""
    if _CACHE.get("act_patched"):
        return
    from concourse import bacc, mybir

    orig = bacc.get_activation_tables
    AF = mybir.ActivationFunctionType
    combo = "natural_log_exp_and_others"

    def patched(arch):
        t = {k: set(v) for k, v in orig(arch).items()}
        if combo in t:
            for name in t:
                if name != combo:
                    t[name] = t[name] - {AF.Ln, AF.Exp}
        return t

    bacc.get_activation_tables = patched
    _CACHE["act_patched"] = True


def _build_nc():
    import concourse.tile as tile
    from concourse import bacc, mybir

    _patch_act_tables()

    nc = bacc.Bacc(
        "TRN2",
        target_bir_lowering=False,
        debug=False,
        enable_asserts=False,
        num_devices=NCORES,
    )
    x_ap = nc.dram_tensor("x", [NS, D, HW], mybir.dt.float32, kind="ExternalInput").ap()
    wt_ap = nc.dram_tensor("wt", [D, K], mybir.dt.float32, kind="ExternalInput").ap()
    cent_ap = nc.dram_tensor(
        "cent", [K, D], mybir.dt.float32, kind="ExternalInput"
    ).ap()
    out_ap = nc.dram_tensor(
        "out", [NS, K, D], mybir.dt.float32, kind="ExternalOutput"
    ).ap()

    with tile.TileContext(nc) as tc:
        with ExitStack() as ctx:
            _body(ctx, tc, out_ap, x_ap, wt_ap, cent_ap)
    nc.compile()
    return nc


def _body(ctx, tc, out_ap, x_ap, wt_ap, cent_ap):
    import concourse.bass as bass
    from concourse import masks, mybir

    nc = tc.nc
    f32 = mybir.dt.float32
    bf16 = mybir.dt.bfloat16
    AF = mybir.ActivationFunctionType
    ALU = mybir.AluOpType
    X_AX = mybir.AxisListType.X

    singles = ctx.enter_context(tc.tile_pool(name="singles", bufs=1))
    xpool = ctx.enter_context(tc.tile_pool(name="xpool", bufs=2))
    xtrpool = ctx.enter_context(tc.tile_pool(name="xtrpool", bufs=4))
    ebpool = ctx.enter_context(tc.tile_pool(name="ebpool", bufs=2))
    sbtpool = ctx.enter_context(tc.tile_pool(name="sbtpool", bufs=4))
    lpool = ctx.enter_context(tc.tile_pool(name="lpool", bufs=2))
    scrpool = ctx.enter_context(tc.tile_pool(name="scrpool", bufs=2))
    smalls = ctx.enter_context(tc.tile_pool(name="smalls", bufs=3))
    tails = ctx.enter_context(tc.tile_pool(name="tails", bufs=1))
    pp_xt = ctx.enter_context(tc.tile_pool(name="pp_xt", bufs=6, space="PSUM"))
    pp_acc = ctx.enter_context(tc.tile_pool(name="pp_acc", bufs=1, space="PSUM"))
    pp_tiny = ctx.enter_context(tc.tile_pool(name="pp_tiny", bufs=1, space="PSUM"))

    def bcast(ap, n):
        # append a step-0 free dim: [..., n] broadcast view
        return bass.AP(tensor=ap.tensor, offset=ap.offset, ap=list(ap.ap) + [[0, n]])

    def mid_bcast(ap, n):
        # [p, f] -> [p, n, f] with step-0 middle dim
        return bass.AP(
            tensor=ap.tensor,
            offset=ap.offset,
            ap=[ap.ap[0], [0, n]] + list(ap.ap[1:]),
        )

    # constants
    ident = singles.tile([128, 128], f32)
    masks.make_identity(nc, ident[:])
    # fused rhs for pass A: [identity | Wt] — one matmul yields [X_c^T | logits]
    identwt = singles.tile([128, 192], f32)
    masks.make_identity(nc, identwt[:, 0:128])
    nc.sync.dma_start(out=identwt[:, 128:192], in_=wt_ap[:])
    wt_s = singles.tile([D, K], f32)
    nc.sync.dma_start(out=wt_s[:], in_=wt_ap[:])
    cent_s = singles.tile([K, D], f32)
    nc.sync.dma_start(out=cent_s[:], in_=cent_ap[:])
    ones_col = singles.tile([K, 1], f32)
    nc.vector.memset(ones_col[:], 1.0)
    ones_row = singles.tile([1, K], f32)
    nc.vector.memset(ones_row[:], 1.0)

    GRP = 2  # fused-matmul chunks per PSUM bank (finer -> more PE runway)
    groups = []
    c0 = 0
    while c0 < NCH:
        groups.append(list(range(c0, min(c0 + GRP, NCH))))
        c0 += GRP

    state = {}  # per-sample live tiles

    def emit_load_and_passA(n):
        xs = xpool.tile([D, HW], f32, tag="xs")
        nc.sync.dma_start(out=xs[:], in_=x_ap[n])

        # [XT | s] per chunk (bf16): cols 0:128 = X_c^T, col 128 = ||x_p||
        xtr = xtrpool.tile([128, NCH, 129], bf16, tag="xtr")
        # softmax numerators e (bf16, contiguous for the 2x reduce)
        et = ebpool.tile([128, NCH, K], bf16, tag="et")
        # sb = e * inv_s/Z — the acc matmul's stationary operand
        sbt = sbtpool.tile([128, NCH, K], bf16, tag="sbt")
        # raw logits stash (bf16)
        lgs = lpool.tile([128, NCH, K], bf16, tag="lgs")
        # XT^2 scratch (bf16 — ss reduce gets the 2x DVE mode)
        x2t = scrpool.tile([128, NCH * 128], bf16, tag="x2t")
        # scaled-logits scratch (bf16)
        slgt = scrpool.tile([128, NCH * K], bf16, tag="slgt")

        for gi, grp in enumerate(groups):
            gn = len(grp)
            # one fused matmul per chunk: out cols 0:128 = X_c^T, 128:192 =
            # logits. 256-col stride keeps each 192-col output in one bank.
            xt_p = pp_xt.tile([128, GRP, 256], f32, tag="xt")
            for j, c in enumerate(grp):
                p0, w = CHUNKS[c]
                x_c = xs[:, p0 : p0 + w]
                nc.tensor.matmul(
                    xt_p[:w, j, 0:192],
                    lhsT=x_c,
                    rhs=identwt[:],
                    start=True,
                    stop=True,
                )
            gc = grp[0]
            # alternate evacuation between DVE and ACT so neither engine's
            # batch work starves the PE's PSUM recycling
            if gi % 2 == 0:
                nc.vector.tensor_copy(
                    xtr[:, gc : gc + gn, 0:128], xt_p[:, 0:gn, 0:128]
                )
                nc.scalar.copy(lgs[:, gc : gc + gn, :], xt_p[:, 0:gn, 128:192])
            else:
                nc.scalar.copy(xtr[:, gc : gc + gn, 0:128], xt_p[:, 0:gn, 0:128])
                nc.vector.tensor_copy(lgs[:, gc : gc + gn, :], xt_p[:, 0:gn, 128:192])

        state[n] = (xs, xtr, et, sbt, lgs, x2t, slgt)

    def emit_scalars(n):
        xs, xtr, et, sbt, lgs, x2t, slgt = state[n]
        ss = smalls.tile([128, NCH], f32, tag="ss")
        zz = smalls.tile([128, NCH], f32, tag="zz")
        is_ = smalls.tile([128, NCH], f32, tag="is")
        tsc = smalls.tile([128, NCH], bf16, tag="tsc")

        # everything split per half-sample: shorter dependency links, and
        # the static per-engine schedule interleaves across samples better
        halves = [(0, NCH // 2), (NCH // 2, NCH)]
        lns = smalls.tile([128, NCH], f32, tag="lns")
        x2vf = x2t[:].rearrange("p (c d) -> p c d", c=NCH)
        for h0, h1 in halves:
            nc.scalar.activation(x2vf[:, h0:h1, :], xtr[:, h0:h1, 0:128], AF.Square)
            nc.vector.tensor_reduce(
                out=ss[:, h0:h1], in_=x2vf[:, h0:h1, :], axis=X_AX, op=ALU.add
            )
            # inv_s = exp(-0.5*ln(ss)); Ln+Exp live in one ACT table set
            nc.scalar.activation(lns[:, h0:h1], ss[:, h0:h1], AF.Ln)
            nc.scalar.activation(is_[:, h0:h1], lns[:, h0:h1], AF.Exp, scale=-0.5)
            # s = ss * inv_s = ||x_p||, into col 128 of each xtr chunk (the
            # acc matmul's rhs column turning sb into sum_sa)
            nc.gpsimd.tensor_tensor(
                out=xtr[:, h0:h1, 128],
                in0=ss[:, h0:h1],
                in1=is_[:, h0:h1],
                op=ALU.mult,
            )

        for h0, h1 in halves:
            slg = slgt[:, h0 * K : h1 * K].rearrange("p (c k) -> p c k", c=h1 - h0)
            nc.gpsimd.tensor_tensor(
                out=slg,
                in0=lgs[:, h0:h1, :],
                in1=bcast(is_[:, h0:h1], K),
                op=ALU.mult,
            )
            nc.scalar.activation(et[:, h0:h1, :], slg, AF.Exp)
            nc.vector.tensor_reduce(
                out=zz[:, h0:h1], in_=et[:, h0:h1, :], axis=X_AX, op=ALU.add
            )
            rr = smalls.tile([128, NCH // 2], f32, tag="rr")
            nc.vector.reciprocal(rr[:], zz[:, h0:h1])
            # t = inv_s / Z
            nc.gpsimd.tensor_tensor(
                out=tsc[:, h0:h1], in0=is_[:, h0:h1], in1=rr[:], op=ALU.mult
            )
            # sb = e * t
            nc.gpsimd.tensor_tensor(
                out=sbt[:, h0:h1, :],
                in0=et[:, h0:h1, :],
                in1=bcast(tsc[:, h0:h1], K),
                op=ALU.mult,
            )

    def emit_passC(n, agg_all, ssa_all):
        xs, xtr, et, sbt, lgs, x2t, slgt = state.pop(n)
        acc_p = pp_acc.tile([K, 129], f32, tag="acc")
        for c, (p0, w) in enumerate(CHUNKS):
            nc.tensor.matmul(
                acc_p[:, :],
                lhsT=sbt[:w, c, :],
                rhs=xtr[:w, c, :],
                start=(c == 0),
                stop=(c == NCH - 1),
            )
        # evacuate: agg = cols 0:128; sum_sa = col 128
        nc.vector.tensor_copy(agg_all[:, n, :], acc_p[:, 0:D])
        nc.scalar.copy(ssa_all[:, n : n + 1], acc_p[:, 128:129])

    # batched across all samples
    agg_all = tails.tile([K, NS, D], f32)
    ssa_all = tails.tile([K, NS], f32)

    # emission order per round: pass A of sample n FIRST (so its PSUM-evac
    # copies sit ahead of batch reduces in the DVE/ACT queues), then the
    # scalar chain of n-1, then the acc matmuls of n-3.
    PIPE = 3
    for n in range(NS):
        emit_load_and_passA(n)
        if n >= 1:
            emit_scalars(n - 1)
        if n >= PIPE:
            emit_passC(n - PIPE, agg_all, ssa_all)
    emit_scalars(NS - 1)
    for n in range(NS - PIPE, NS):
        emit_passC(n, agg_all, ssa_all)

    # ---- batched tail over all samples ----
    vl = tails.tile([K, NS, D], f32)
    vsq = tails.tile([K, NS * D], f32)
    q = tails.tile([K, NS], f32)
    qm = tails.tile([K, NS], f32)
    isq = tails.tile([K, NS], f32)
    isq2 = tails.tile([K, NS], f32)
    u = tails.tile([K, NS], f32)
    gisr = tails.tile([1, NS], f32)
    gb = tails.tile([K, NS], f32)
    sall = tails.tile([K, NS], f32)
    vf = tails.tile([K, NS, D], f32)

    # vl = agg - ssa * cent
    nc.gpsimd.tensor_tensor(
        out=vl[:], in0=bcast(ssa_all[:], D), in1=mid_bcast(cent_s[:], NS), op=ALU.mult
    )
    nc.vector.tensor_tensor(out=vl[:], in0=agg_all[:], in1=vl[:], op=ALU.subtract)
    # q = rowsum(vl^2) per (k, n)
    vsqv = vsq[:].rearrange("k (n d) -> k n d", n=NS)
    nc.scalar.activation(vsqv, vl[:], AF.Square)
    nc.vector.tensor_reduce(out=q[:], in_=vsqv, axis=X_AX, op=ALU.add)
    nc.vector.tensor_scalar_max(qm[:], q[:], 1e-24)
    lq = tails.tile([K, NS], f32)
    nc.scalar.activation(lq[:], qm[:], AF.Ln)
    nc.scalar.activation(isq[:], lq[:], AF.Exp, scale=-0.5)
    # g = sum_k q_k * isq_k^2  (per sample)
    nc.vector.tensor_tensor(out=isq2[:], in0=isq[:], in1=isq[:], op=ALU.mult)
    nc.vector.tensor_tensor(out=u[:], in0=q[:], in1=isq2[:], op=ALU.mult)
    g_p = pp_tiny.tile([NS, 1], f32, tag="tiny")
    nc.tensor.matmul(g_p[:], lhsT=u[:], rhs=ones_col[:], start=True, stop=True)
    # gis = g^-0.5 -> transpose to a row -> broadcast over k partitions
    gm = tails.tile([NS, 1], f32)
    nc.vector.tensor_scalar_max(gm[:], g_p[:], 1e-24)
    gis = tails.tile([NS, 1], f32)
    lgm = tails.tile([NS, 1], f32)
    nc.scalar.activation(lgm[:], gm[:], AF.Ln)
    nc.scalar.activation(gis[:], lgm[:], AF.Exp, scale=-0.5)
    gr_p = pp_tiny.tile([1, NS], f32, tag="tiny")
    nc.tensor.matmul(gr_p[:], lhsT=gis[:], rhs=ident[:NS, :NS], start=True, stop=True)
    nc.vector.tensor_copy(gisr[:], gr_p[:])
    gb_p = pp_tiny.tile([K, NS], f32, tag="tiny")
    nc.tensor.matmul(gb_p[:], lhsT=ones_row[:], rhs=gisr[:], start=True, stop=True)
    nc.vector.tensor_copy(gb[:], gb_p[:])
    # s = isq * gb; vf = vl * s
    nc.vector.tensor_tensor(out=sall[:], in0=isq[:], in1=gb[:], op=ALU.mult)
    nc.gpsimd.tensor_tensor(out=vf[:], in0=vl[:], in1=bcast(sall[:], D), op=ALU.mult)
    nc.sync.dma_start(out=out_ap.rearrange("n k d -> k n d"), in_=vf[:])


def kernel(x, conv_w, centroids):
    from concourse.bass_utils import run_bass_kernel_spmd

    if "nc" not in _CACHE:
        _CACHE["nc"] = _build_nc()
    nc = _CACHE["nc"]

    x = np.ascontiguousarray(np.asarray(x, dtype=np.float32).reshape(N, D, HW))
    wt = np.ascontiguousarray(np.asarray(conv_w, dtype=np.float32).T)
    cent = np.ascontiguousarray(np.asarray(centroids, dtype=np.float32))
    in_maps = [
        {"x": x[i * NS : (i + 1) * NS], "wt": wt, "cent": cent} for i in range(NCORES)
    ]
    res = run_bass_kernel_spmd(nc, in_maps, core_ids=list(range(NCORES))).results
    out = np.concatenate([r["out"].reshape(NS, K * D) for r in res], axis=0)
    return out


if __name__ == "__main__":
    rng = np.random.default_rng(0)
    xs = rng.standard_normal((N, D, 60, 80), dtype=np.float32)
    cw = (rng.standard_normal((K, D)) * 0.1).astype(np.float32)
    ct = rng.random((K, D), dtype=np.float32)
    o = kernel(x=xs, conv_w=cw, centroids=ct)
    print("kernel out", o.shape, o.dtype, np.abs(o).max())
